# revision 10
# baseline (speedup 1.0000x reference)
"""GAT-DGG 2-layer kernel for 8 Trainium2 NeuronCores.

Math (per attention map, per GAT layer):
  reference: A = full(-1e20).at[src,tgt].set(e) * adj;  att = softmax(A, axis=1)
  Non-edge entries of A become 0 (not -inf), so P = exp(A) has value 1 at
  non-edges and exp(e_ij) at edges, where e_ij = leaky_relu(s_i + t_j),
  s = h@a1, t = h@a2.  Hence with alpha=0.2:
     exp(leaky(v)) = exp(a*v) * max(exp((1-a)*v), 1)        (v = s_i + t_j)
  both exp(a*v) and exp((1-a)*v) are rank-1 in (i,j).  Define
     M_ij  = max(e1s_i * e1tp_j, eta_j)   with e1s = exp((1-a)s),
             e1tp = exp(t), eta = exp(a*t)          [e1tp = exp((1-a)t)*eta]
  then  P = 1 + adj .* (esa_i * M - 1),  esa = exp(a*s), and for any Y:
     (P @ Y)_i = colsum(Y) + esa_i * [(adj.*M) @ Y]_i - [adj @ Y]_i
  row-sums (softmax Z) come from Y's ones-column.

Sharding: rows (target dim of softmax) split 512/core; adj passed
pre-transposed+bf16 per core so the big matmul (adj.*M)^T streams naturally.
Only inter-core exchange: AllGather of layer-1 node features h2 [512,12].
"""
import sys

sys.path.insert(0, "/opt/trn_rl_repo")

import numpy as np
import ml_dtypes

import concourse.bass as bass
import concourse.tile as tile
from concourse import bacc, mybir
from concourse.bass_utils import run_bass_kernel_spmd
from concourse.masks import make_identity

N = 4096
R = 512          # rows per core
NC = 8           # cores
F = 32           # feature dim
H = 8            # heads
C2 = 10          # classes
ALPHA = 0.2
F32 = mybir.dt.float32
BF16 = mybir.dt.bfloat16
AO = mybir.AluOpType
AF = mybir.ActivationFunctionType

_CACHE = {}


def _build():
    nc = bacc.Bacc("TRN2", target_bir_lowering=False, debug=False, num_devices=NC)
    dt = lambda name, shape, d, kind: nc.dram_tensor(name, shape, d, kind=kind).ap()
    I = "ExternalInput"
    xTe_f_d = dt("xTe_f", [F + 1, N], F32, I)          # [33, 4096] x.T + ones row
    xTe_bf_d = dt("xTe_bf", [F + 1, N], BF16, I)
    xTeo_d = dt("xTeo", [F + 1, R], F32, I)            # own columns slice
    xe_d = dt("xe", [N, F + 1], BF16, I)               # x + ones col
    adjT_d = dt("adjT", [N, R], BF16, I)               # adj.T[:, own rows]
    Wcat_f_d = dt("Wcat_f", [F + 1, H * (F + 1)], F32, I)
    Wcat_bf_d = dt("Wcat_bf", [F + 1, H * (F + 1)], BF16, I)
    Wtst_d = dt("Wtst", [F + 1, 2 * H], F32, I)        # cols 0-7 s-heads, 8-15 t-heads
    wout_d = dt("wout", [F, H, C2], F32, I)            # W_out [256,10] -> [32, 8, 10]
    aout_d = dt("aout", [C2, 2], F32, I)
    ball_d = dt("ball", [F, H], F32, I)                # b_heads.T
    bout_d = dt("bout", [C2 + 1, 1], F32, I)
    negcw_d = dt("negcw", [1, C2], F32, I)             # -W_out.sum(0)
    sel8bf_d = dt("sel8bf", [H, H * 128], BF16, I)     # one-hot head selector
    sel8f_d = dt("sel8f", [H, H * 128], F32, I)
    out_d = dt("out", [R, C2], F32, "ExternalOutput")

    JT = N // 128          # 32 j-tiles
    IT = R // 128          # 4 i-tiles
    FC = F + 1             # 33
    CHUNK = 8              # j-tiles per TS/TT chunk
    NCH = JT // CHUNK      # 4 chunks

    from contextlib import ExitStack
    with tile.TileContext(nc) as tc, ExitStack() as ctx:
        cst = ctx.enter_context(tc.tile_pool(name="cst", bufs=1))
        wrk = ctx.enter_context(tc.tile_pool(name="wrk", bufs=2))
        epi = ctx.enter_context(tc.tile_pool(name="epi", bufs=1))
        ps = ctx.enter_context(tc.tile_pool(name="ps", bufs=3, space="PSUM"))
        ps1 = ctx.enter_context(tc.tile_pool(name="ps1", bufs=1, space="PSUM"))
        dram = ctx.enter_context(tc.tile_pool(name="dram", bufs=1, space="DRAM"))

        # ---------- constants ----------
        xTe_f = cst.tile([FC, N], F32)
        nc.sync.dma_start(xTe_f, xTe_f_d)
        xTe_bf = cst.tile([FC, N], BF16)
        nc.sync.dma_start(xTe_bf, xTe_bf_d)
        xTeo = cst.tile([FC, R], F32)
        nc.sync.dma_start(xTeo, xTeo_d)
        xe_sb = cst.tile([128, JT, FC], BF16)
        nc.sync.dma_start(xe_sb, xe_d.rearrange("(t p) c -> p t c", p=128))
        adjT = cst.tile([128, JT, R], BF16)
        nc.sync.dma_start(adjT, adjT_d.rearrange("(t p) i -> p t i", p=128))
        Wcat_f = cst.tile([FC, H * FC], F32)
        nc.sync.dma_start(Wcat_f, Wcat_f_d)
        Wcat_bf = cst.tile([FC, H * FC], BF16)
        nc.sync.dma_start(Wcat_bf, Wcat_bf_d)
        Wtst = cst.tile([FC, 2 * H], F32)
        nc.sync.dma_start(Wtst, Wtst_d)
        wout = cst.tile([F, H, C2], F32)
        nc.sync.dma_start(wout, wout_d)
        aout = cst.tile([C2, 2], F32)
        nc.sync.dma_start(aout, aout_d)
        ball = cst.tile([F, H], F32)
        nc.sync.dma_start(ball, ball_d)
        bout = cst.tile([C2 + 1, 1], F32)
        nc.sync.dma_start(bout, bout_d)
        negcw = cst.tile([1, C2], F32)
        nc.sync.dma_start(negcw, negcw_d)
        sel8bf = cst.tile([H, H * 128], BF16)
        nc.sync.dma_start(sel8bf, sel8bf_d)
        sel8f = cst.tile([H, H * 128], F32)
        nc.sync.dma_start(sel8f, sel8f_d)

        ident = cst.tile([128, 128], F32)
        make_identity(nc, ident)
        ones1bf = cst.tile([1, 128], BF16)
        nc.vector.memset(ones1bf, 1.0)
        ones1f = cst.tile([1, 128], F32)
        nc.vector.memset(ones1f, 1.0)
        onesRow = cst.tile([1, R], F32)
        nc.vector.memset(onesRow, 1.0)
        ones128bf = cst.tile([128, 1], BF16)
        nc.vector.memset(ones128bf, 1.0)

        # ---------- stage A: per-node vectors ----------
        # stT[j, :] = [s_h(j) for h] ++ [t_h(j) for h]   (all nodes)
        st_sb = cst.tile([128, JT, 2 * H], F32)
        for t in range(JT):
            p = ps.tile([128, 2 * H], F32, tag="scratch")
            nc.tensor.matmul(p, xTe_f[:, t * 128:(t + 1) * 128], Wtst,
                             start=True, stop=True)
            nc.vector.tensor_copy(st_sb[:, t, :], p)
        # exp vectors over all nodes (t-halves)
        e1tp = cst.tile([128, JT, H], F32)   # exp(t)
        eta = cst.tile([128, JT, H], F32)    # exp(alpha*t)
        nc.scalar.activation(e1tp, st_sb[:, :, H:2 * H], AF.Exp)
        nc.scalar.activation(eta, st_sb[:, :, H:2 * H], AF.Exp, scale=ALPHA)

        # own rows: s in row layout [16, 512] via transpose
        sto = cst.tile([128, IT, 2 * H], F32)
        for t in range(IT):
            p = ps.tile([128, 2 * H], F32, tag="scratch")
            nc.tensor.matmul(p, xTeo[:, t * 128:(t + 1) * 128], Wtst,
                             start=True, stop=True)
            nc.vector.tensor_copy(sto[:, t, :], p)
        stRow = cst.tile([2 * H, IT, 128], F32)
        for t in range(IT):
            p = ps.tile([2 * H, 128], F32, tag="scratch")
            nc.tensor.transpose(p, sto[:, t, :], ident)
            nc.vector.tensor_copy(stRow[:, t, :], p)
        stRowV = stRow.rearrange("c t p -> c (t p)")
        esaRow = cst.tile([H, R], F32)
        nc.scalar.activation(esaRow, stRowV[0:H, :], AF.Exp, scale=ALPHA)
        e1sRow = cst.tile([H, R], BF16)
        nc.scalar.activation(e1sRow, stRowV[0:H, :], AF.Exp, scale=1.0 - ALPHA)

        # xsum_e = sum_j xe[j]  -> Htot per-head col layout [33, 8]
        ps_xs = ps1.tile([FC, 1], F32, tag="acc_small")
        for t in range(JT):
            nc.tensor.matmul(ps_xs, xe_sb[:, t, :], ones128bf,
                             start=(t == 0), stop=(t == JT - 1))
        xsum = cst.tile([FC, 1], F32)
        nc.vector.tensor_copy(xsum, ps_xs)
        HtT = cst.tile([FC, H], F32)
        for h in range(H):
            p = ps.tile([FC, 1], F32, tag="scratch")
            nc.tensor.matmul(p, Wcat_f[:, h * FC:(h + 1) * FC], xsum,
                             start=True, stop=True)
            nc.vector.tensor_copy(HtT[:, h:h + 1], p)

        # h_sb[j, h, :] = [h_h(j) | 1]  (bf16, all nodes)
        h_sb = cst.tile([128, JT, H, FC], BF16)
        for t in range(JT):
            p = ps.tile([128, H * F], F32, tag="scratch")
            nc.tensor.matmul(
                p, xTe_bf[:, t * 128:(t + 1) * 128],
                Wcat_bf.rearrange("k (h c) -> k h c", h=H)[:, :, 0:F],
                start=True, stop=True)
            nc.vector.tensor_copy(
                h_sb[:, t, :, 0:F],
                p.rearrange("p (h c) -> p h c", h=H))
            nc.vector.memset(h_sb[:, t, :, F], 1.0)

        # D = adj @ [x|1]  (own rows), .T layout [33, 512]
        ps_D = ps1.tile([FC, R], F32, tag="acc_D")
        for t in range(JT):
            nc.tensor.matmul(ps_D, xe_sb[:, t, :], adjT[:, t, :],
                             start=(t == 0), stop=(t == JT - 1))
        D_sb = cst.tile([FC, R], F32)
        nc.vector.tensor_copy(D_sb, ps_D)

        # ---------- layer-1 heads ----------
        ps_h2 = ps1.tile([C2, R], F32, tag="acc_h2")
        for h in range(H):
            # broadcast e1s row -> [128, 512] bf16
            pb = ps.tile([128, R], F32, tag="scratch")
            nc.tensor.matmul(pb, sel8bf[:, h * 128:(h + 1) * 128], e1sRow,
                             start=True, stop=True)
            e1sB = wrk.tile([128, R], BF16, tag="e1sB")
            nc.vector.tensor_copy(e1sB, pb)

            ps_o1 = ps1.tile([FC, R], F32, tag=f"acc_o_{h % 2}")
            for jc in range(NCH):
                m_buf = wrk.tile([128, CHUNK, R], BF16, tag="m_buf")
                B_buf = wrk.tile([128, CHUNK, R], BF16, tag="B_buf")
                for jl in range(CHUNK):
                    jt = jc * CHUNK + jl
                    nc.vector.tensor_scalar(
                        m_buf[:, jl, :], e1sB,
                        e1tp[:, jt, h:h + 1], eta[:, jt, h:h + 1],
                        op0=AO.mult, op1=AO.max)
                nc.vector.tensor_tensor(
                    B_buf.rearrange("p a b -> p (a b)"),
                    m_buf.rearrange("p a b -> p (a b)"),
                    adjT[:, jc * CHUNK:(jc + 1) * CHUNK, :]
                        .rearrange("p a b -> p (a b)"),
                    op=AO.mult)
                for jl in range(CHUNK):
                    jt = jc * CHUNK + jl
                    nc.tensor.matmul(ps_o1, h_sb[:, jt, h, :], B_buf[:, jl, :],
                                     start=(jt == 0), stop=(jt == JT - 1))

            # epilogue
            ps_aY = ps.tile([FC, R], F32, tag="scratch")
            nc.tensor.matmul(ps_aY, Wcat_f[:, h * FC:(h + 1) * FC], D_sb,
                             start=True, stop=True)
            ps_e = ps.tile([FC, R], F32, tag="scratch")
            nc.tensor.matmul(ps_e, sel8f[:, h * 128:h * 128 + FC], esaRow,
                             start=True, stop=True)
            esa_sb = epi.tile([FC, R], F32, tag="esa_sb")
            nc.vector.tensor_copy(esa_sb, ps_e)
            o1e = epi.tile([FC, R], F32, tag="o1e")
            nc.vector.tensor_tensor(o1e, ps_o1, esa_sb, op=AO.mult)
            q = epi.tile([FC, R], F32, tag="q")
            nc.vector.tensor_tensor(q, o1e, ps_aY, op=AO.subtract)
            q2 = epi.tile([FC, R], F32, tag="q2")
            nc.vector.tensor_scalar(q2, q, HtT[:, h:h + 1], None, op0=AO.add)
            rz = epi.tile([1, R], F32, tag="rz")
            nc.vector.reciprocal(rz, q2[F:FC, :])
            ps_r = ps.tile([F, R], F32, tag="scratch")
            nc.tensor.matmul(ps_r, ones1f[:, 0:F], rz, start=True, stop=True)
            xc = epi.tile([F, R], F32, tag="xc")
            nc.vector.tensor_tensor(xc, q2[0:F, :], ps_r, op=AO.mult)
            # ELU(xc + b) + 1 = relu(z) + exp(min(z,0)),  z = xc + b
            r = epi.tile([F, R], F32, tag="r")
            nc.vector.tensor_scalar(r, xc, ball[:, h:h + 1], 0.0,
                                    op0=AO.add, op1=AO.max)
            d = epi.tile([F, R], F32, tag="d")
            nc.vector.tensor_tensor(d, xc, r, op=AO.subtract)
            en = epi.tile([F, R], F32, tag="en")
            nc.scalar.activation(en, d, AF.Exp, bias=ball[:, h:h + 1])
            el = epi.tile([F, R], F32, tag="el")
            nc.vector.tensor_tensor(el, r, en, op=AO.add)
            nc.tensor.matmul(ps_h2, wout[:, h, :], el,
                             start=(h == 0), stop=False)
        # -1 correction for the (elu+1) trick
        nc.tensor.matmul(ps_h2, negcw, onesRow, start=False, stop=True)

        # ---------- exchange h2 ----------
        h2T = cst.tile([C2, R], F32)
        nc.vector.tensor_copy(h2T, ps_h2)
        ps_st2 = ps.tile([2, R], F32, tag="scratch")
        nc.tensor.matmul(ps_st2, aout, h2T, start=True, stop=True)
        e1s2Row = cst.tile([1, R], BF16)
        nc.scalar.activation(e1s2Row, ps_st2[0:1, :], AF.Exp, scale=1.0 - ALPHA)
        esa2Row = cst.tile([1, R], F32)
        nc.scalar.activation(esa2Row, ps_st2[0:1, :], AF.Exp, scale=ALPHA)
        st2_sb = cst.tile([2, R], F32)
        nc.vector.tensor_copy(st2_sb, ps_st2)

        gin = cst.tile([128, IT, C2 + 2], F32)
        for t in range(IT):
            p = ps.tile([128, C2], F32, tag="scratch")
            nc.tensor.transpose(p, h2T[:, t * 128:(t + 1) * 128], ident[0:C2, 0:C2])
            nc.vector.tensor_copy(gin[:, t, 0:C2], p)
            p2 = ps.tile([128, 2], F32, tag="scratch")
            nc.tensor.transpose(p2, st2_sb[:, t * 128:(t + 1) * 128], ident[0:2, 0:2])
            nc.vector.tensor_copy(gin[:, t, C2:C2 + 2], p2)
        agi = dram.tile([R, C2 + 2], F32)
        nc.sync.dma_start(agi.rearrange("(t p) c -> p t c", p=128), gin)
        ago = dram.tile([N, C2 + 2], F32)
        nc.gpsimd.collective_compute(
            "AllGather", AO.bypass, replica_groups=[list(range(NC))],
            ins=[agi.opt()], outs=[ago.opt()])
        g_sb = cst.tile([128, JT, C2 + 2], F32)
        nc.sync.dma_start(g_sb, ago.rearrange("(t p) c -> p t c", p=128))

        # ---------- layer-2 ----------
        e1tp2 = cst.tile([128, JT], F32)
        nc.scalar.activation(e1tp2, g_sb[:, :, C2 + 1], AF.Exp)
        eta2 = cst.tile([128, JT], F32)
        nc.scalar.activation(eta2, g_sb[:, :, C2 + 1], AF.Exp, scale=ALPHA)
        l2_sb = cst.tile([128, JT, FC], BF16)
        for t in range(JT):
            nc.vector.tensor_copy(l2_sb[:, t, 0:C2], g_sb[:, t, 0:C2])
        nc.vector.memset(l2_sb[:, :, C2:F], 0.0)
        nc.vector.memset(l2_sb[:, :, F], 1.0)

        ps_H2t = ps1.tile([FC, 1], F32, tag="acc_small")
        ps_D2 = ps1.tile([FC, R], F32, tag="acc_D")
        for t in range(JT):
            nc.tensor.matmul(ps_H2t, l2_sb[:, t, :], ones128bf,
                             start=(t == 0), stop=(t == JT - 1))
            nc.tensor.matmul(ps_D2, l2_sb[:, t, :], adjT[:, t, :],
                             start=(t == 0), stop=(t == JT - 1))
        H2t = cst.tile([FC, 1], F32)
        nc.vector.tensor_copy(H2t, ps_H2t)

        pb = ps.tile([128, R], F32, tag="scratch")
        nc.tensor.matmul(pb, ones1bf, e1s2Row, start=True, stop=True)
        e1sB2 = wrk.tile([128, R], BF16, tag="e1sB")
        nc.vector.tensor_copy(e1sB2, pb)
        ps_o2 = ps1.tile([FC, R], F32, tag="acc_o_0")
        for jc in range(NCH):
            m_buf = wrk.tile([128, CHUNK, R], BF16, tag="m_buf")
            B_buf = wrk.tile([128, CHUNK, R], BF16, tag="B_buf")
            for jl in range(CHUNK):
                jt = jc * CHUNK + jl
                nc.vector.tensor_scalar(
                    m_buf[:, jl, :], e1sB2,
                    e1tp2[:, jt:jt + 1], eta2[:, jt:jt + 1],
                    op0=AO.mult, op1=AO.max)
            nc.vector.tensor_tensor(
                B_buf.rearrange("p a b -> p (a b)"),
                m_buf.rearrange("p a b -> p (a b)"),
                adjT[:, jc * CHUNK:(jc + 1) * CHUNK, :]
                    .rearrange("p a b -> p (a b)"),
                op=AO.mult)
            for jl in range(CHUNK):
                jt = jc * CHUNK + jl
                nc.tensor.matmul(ps_o2, l2_sb[:, jt, :], B_buf[:, jl, :],
                                 start=(jt == 0), stop=(jt == JT - 1))

        ps_e2 = ps.tile([FC, R], F32, tag="scratch")
        nc.tensor.matmul(ps_e2, ones1f[:, 0:FC], esa2Row, start=True, stop=True)
        esa2_sb = epi.tile([FC, R], F32, tag="esa2_sb")
        nc.vector.tensor_copy(esa2_sb, ps_e2)
        o2e = epi.tile([FC, R], F32, tag="o2e")
        nc.vector.tensor_tensor(o2e, ps_o2, esa2_sb, op=AO.mult)
        q_2 = epi.tile([FC, R], F32, tag="q_2")
        nc.vector.tensor_tensor(q_2, o2e, ps_D2, op=AO.subtract)
        q2_2 = epi.tile([FC, R], F32, tag="q2_2")
        nc.vector.tensor_scalar(q2_2, q_2, H2t, None, op0=AO.add)
        rz2 = epi.tile([1, R], F32, tag="rz")
        nc.vector.reciprocal(rz2, q2_2[F:FC, :])
        ps_r2 = ps.tile([C2, R], F32, tag="scratch")
        nc.tensor.matmul(ps_r2, ones1f[:, 0:C2], rz2, start=True, stop=True)
        xc2 = epi.tile([C2, R], F32, tag="xc2")
        nc.vector.tensor_tensor(xc2, q2_2[0:C2, :], ps_r2, op=AO.mult)
        xcb = epi.tile([C2, R], F32, tag="xcb")
        nc.vector.tensor_scalar(xcb, xc2, bout[0:C2, :], None, op0=AO.add)

        # log_softmax over classes (transpose to [i, c] layout first)
        out_sb = cst.tile([128, IT, C2], F32)
        for t in range(IT):
            p = ps.tile([128, C2], F32, tag="scratch")
            nc.tensor.transpose(p, xcb[:, t * 128:(t + 1) * 128], ident[0:C2, 0:C2])
            ot = epi.tile([128, C2], F32, tag="ot")
            nc.vector.tensor_copy(ot, p)
            mx = epi.tile([128, 1], F32, tag="mx")
            nc.vector.tensor_reduce(mx, ot, axis=mybir.AxisListType.X, op=AO.max)
            zm = epi.tile([128, C2], F32, tag="zm")
            nc.vector.tensor_scalar(zm, ot, mx, None, op0=AO.subtract)
            ex = epi.tile([128, C2], F32, tag="ex")
            sm = epi.tile([128, 1], F32, tag="sm")
            nc.scalar.activation(ex, zm, AF.Exp, accum_out=sm)
            ln = epi.tile([128, 1], F32, tag="ln")
            nc.scalar.activation(ln, sm, AF.Ln)
            nc.vector.tensor_scalar(out_sb[:, t, :], zm, ln, None,
                                    op0=AO.subtract)
        nc.sync.dma_start(out_d.rearrange("(t p) c -> p t c", p=128), out_sb)

    nc.compile()
    return nc


def _prep(inputs):
    x = np.asarray(inputs["x"], np.float32)
    adj = np.asarray(inputs["adj"], np.float32)
    W_heads = np.asarray(inputs["W_heads"], np.float32)
    a_heads = np.asarray(inputs["a_heads"], np.float32)
    b_heads = np.asarray(inputs["b_heads"], np.float32)
    W_out = np.asarray(inputs["W_out"], np.float32)
    a_out = np.asarray(inputs["a_out"], np.float32)
    b_out = np.asarray(inputs["b_out"], np.float32)

    ones_n = np.ones((1, N), np.float32)
    xTe = np.concatenate([x.T, ones_n], 0)                      # [33, 4096]
    xe = np.concatenate([x, np.ones((N, 1), np.float32)], 1)    # [4096, 33]
    FC = F + 1
    Wcat = np.zeros((FC, H * FC), np.float32)
    for h in range(H):
        Wcat[0:F, h * FC:h * FC + F] = W_heads[h]
        Wcat[F, h * FC + F] = 1.0
    Wtst = np.zeros((FC, 2 * H), np.float32)
    for h in range(H):
        Wtst[0:F, h] = W_heads[h] @ a_heads[h, 0:F, 0]
        Wtst[0:F, H + h] = W_heads[h] @ a_heads[h, F:2 * F, 0]
    wout = np.ascontiguousarray(W_out.reshape(H, F, C2).transpose(1, 0, 2))
    aout = np.stack([a_out[0:C2, 0], a_out[C2:2 * C2, 0]], 1)   # [10, 2]
    ball = np.ascontiguousarray(b_heads.T)                      # [32, 8]
    bout = np.concatenate([b_out, [0.0]]).reshape(C2 + 1, 1).astype(np.float32)
    negcw = (-W_out.sum(0)).reshape(1, C2).astype(np.float32)
    sel8 = np.zeros((H, H * 128), np.float32)
    for h in range(H):
        sel8[h, h * 128:(h + 1) * 128] = 1.0
    adjT_bf = adj.T.astype(ml_dtypes.bfloat16)

    bf = lambda a: np.ascontiguousarray(a).astype(ml_dtypes.bfloat16)
    f = np.ascontiguousarray
    in_maps = []
    for k in range(NC):
        sl = slice(k * R, (k + 1) * R)
        in_maps.append({
            "xTe_f": f(xTe), "xTe_bf": bf(xTe), "xTeo": f(xTe[:, sl]),
            "xe": bf(xe), "adjT": f(adjT_bf[:, sl]),
            "Wcat_f": f(Wcat), "Wcat_bf": bf(Wcat), "Wtst": f(Wtst),
            "wout": f(wout), "aout": f(aout), "ball": f(ball),
            "bout": f(bout), "negcw": f(negcw),
            "sel8bf": bf(sel8), "sel8f": f(sel8),
        })
    return in_maps


def kernel(**inputs):
    if "nc" not in _CACHE:
        _CACHE["nc"] = _build()
    in_maps = _prep(inputs)
    res = run_bass_kernel_spmd(_CACHE["nc"], in_maps, core_ids=list(range(NC)))
    return np.concatenate([res.results[k]["out"] for k in range(NC)], 0)
